# revision 8
# baseline (speedup 1.0000x reference)
import sys

sys.path.insert(0, "/opt/trn_rl_repo")

import numpy as np  # noqa: E402
import ml_dtypes  # noqa: E402

import concourse.bacc as bacc  # noqa: E402
import concourse.mybir as mybir  # noqa: E402
import concourse.tile as tile  # noqa: E402
from contextlib import ExitStack  # noqa: E402

# Problem constants (hardcoded per contract)
B, NQ, NK, D, H, DFF = 4, 1024, 1024, 1024, 16, 4096
DH = D // H            # 64
P = 128
R = 512                # query rows per core (NQ split in 2)
DT = D // P            # 8 feature chunks
FT = DFF // P          # 32 ffn chunks
SCALE = float(1.0 / np.sqrt(np.float32(D)))
EPS = 1e-5
NCORES = 8

f32 = mybir.dt.float32
f32r = mybir.dt.float32r
bf16 = mybir.dt.bfloat16
AF = mybir.ActivationFunctionType
OP = mybir.AluOpType

_CACHE = {}


def _build():
    nc = bacc.Bacc("TRN2", target_bir_lowering=False, debug=False)

    # ---- DRAM I/O (per-core shards; activations are FEATURE-MAJOR i.e. transposed)
    qt = nc.dram_tensor("qt", [D, R], f32, kind="ExternalInput")[:]    # Q[b, rows].T
    kt = nc.dram_tensor("kt", [D, NK], f32, kind="ExternalInput")[:]   # K[b].T
    wq = nc.dram_tensor("wq", [D, D], f32, kind="ExternalInput")[:]
    wk = nc.dram_tensor("wk", [D, D], f32, kind="ExternalInput")[:]
    wv = nc.dram_tensor("wv", [D, D], f32, kind="ExternalInput")[:]
    wo = nc.dram_tensor("wo", [D, D], bf16, kind="ExternalInput")[:]
    w1 = nc.dram_tensor("w1", [D, DFF], bf16, kind="ExternalInput")[:]
    w2 = nc.dram_tensor("w2", [DFF, D], bf16, kind="ExternalInput")[:]
    vecs = nc.dram_tensor("vecs", [P, 96], f32, kind="ExternalInput")[:]
    bv = nc.dram_tensor("bv", [D], f32, kind="ExternalInput")[:]
    out = nc.dram_tensor("out", [D, R], f32, kind="ExternalOutput")[:]

    MM = nc.tensor.matmul

    with tile.TileContext(nc) as tc, ExitStack() as ctx:
        const = ctx.enter_context(tc.tile_pool(name="const", bufs=1))
        act = ctx.enter_context(tc.tile_pool(name="act", bufs=1))
        wp = ctx.enter_context(tc.tile_pool(name="wp", bufs=2))
        ap_ = ctx.enter_context(tc.tile_pool(name="aexp", bufs=2))
        tmp = ctx.enter_context(tc.tile_pool(name="tmp", bufs=1))
        sm = ctx.enter_context(tc.tile_pool(name="sm", bufs=1))
        ppe = ctx.enter_context(tc.tile_pool(name="ppe", bufs=2, space="PSUM"))
        ppa = ctx.enter_context(tc.tile_pool(name="ppa", bufs=2, space="PSUM"))
        ppb = ctx.enter_context(tc.tile_pool(name="ppb", bufs=2, space="PSUM"))

        # ---- bias/gain vectors, packed host-side as [128, 96] (feature d = t*128+p)
        vecs_sb = const.tile([P, 96], f32)
        nc.sync.dma_start(vecs_sb, vecs)
        bqs = vecs_sb[:, 0:8]
        bks = vecs_sb[:, 8:16]
        bos = vecs_sb[:, 16:24]
        b2s = vecs_sb[:, 24:32]
        g0s = vecs_sb[:, 32:40]
        be0s = vecs_sb[:, 40:48]
        g1s = vecs_sb[:, 48:56]
        be1s = vecs_sb[:, 56:64]
        b1s = vecs_sb[:, 64:96]

        ones_cb = const.tile([P, 1], bf16)      # stats matmul lhsT (bf16)
        nc.vector.memset(ones_cb, 1.0)
        ones_row = const.tile([1, P], f32)      # broadcast matmul lhsT (exact f32)
        nc.vector.memset(ones_row, 1.0)
        zeros_col = const.tile([P, 1], f32)
        nc.vector.memset(zeros_col, 0.0)
        eps_t = const.tile([1, 1], f32)
        nc.vector.memset(eps_t, EPS)

        # bv broadcast to all partitions: [128, D] (bias along the free dim of row-major v)
        bv_row = const.tile([1, D], f32)
        nc.sync.dma_start(bv_row, bv[None, :])
        bvb = const.tile([P, D], f32)
        for hf in range(2):
            pt = ppb.tile([P, 512], f32, tag="pb")
            MM(pt, ones_row, bv_row[:, hf * 512:(hf + 1) * 512], start=True, stop=True)
            nc.vector.tensor_copy(bvb[:, hf * 512:(hf + 1) * 512], pt)

        # ---- persistent activations (tags reused across disjoint lifetimes)
        QT = act.tile([P, DT, R], f32r, tag="a16")       # -> XT -> OUTT
        nc.sync.dma_start(QT, qt.rearrange("(t p) r -> p t r", p=P).bitcast(f32r))

        qT = act.tile([P, DT, R], bf16, tag="b16")       # -> X0 -> X1
        kT = act.tile([P, DT, NK], bf16, tag="kt16")     # -> HT
        v_sb = act.tile([P, DT, H, DH + 1], bf16, tag="v17")
        nc.vector.memset(v_sb[:, :, :, DH:DH + 1], 1.0)

        # ---- projections: stream K^T in halves; weights in quarter-slices
        wre = {"wq": wq.rearrange("(t p) f -> p t f", p=P).bitcast(f32r),
               "wk": wk.rearrange("(t p) f -> p t f", p=P).bitcast(f32r),
               "wv": wv.rearrange("(t p) f -> p t f", p=P).bitcast(f32r),
               "wo": wo.rearrange("(t p) f -> p t f", p=P)}

        def wslice(name, q, dt_):
            wt = wp.tile([P, DT, 256], dt_, tag="w")
            nc.sync.dma_start(wt, wre[name][:, :, q * 256:(q + 1) * 256])
            return wt

        # q projection: qT[do, r] = sum_di Wq[di, do] * QT[di, r]  (+bq)
        for q in range(4):
            wt = wslice("wq", q, f32r)
            for dof in range(2):
                do = q * 2 + dof
                ps = ppa.tile([P, 512], f32, tag="acc")
                for di in range(DT):
                    MM(ps, wt[:, di, dof * 128:(dof + 1) * 128], QT[:, di, :],
                       start=di == 0, stop=di == DT - 1)
                nc.vector.tensor_scalar_add(qT[:, do, :], ps, bqs[:, do:do + 1])

        for half in range(2):
            KTh = act.tile([P, DT, 512], f32r, tag="kth")
            nc.sync.dma_start(KTh, kt.rearrange("(t p) r -> p t r", p=P)[:, :, half * 512:(half + 1) * 512].bitcast(f32r))
            # k projection for this half of the keys
            for q in range(4):
                wt = wslice("wk", q, f32r)
                for dof in range(2):
                    do = q * 2 + dof
                    ps = ppa.tile([P, 512], f32, tag="acc")
                    for di in range(DT):
                        MM(ps, wt[:, di, dof * 128:(dof + 1) * 128], KTh[:, di, :],
                           start=di == 0, stop=di == DT - 1)
                    nc.vector.tensor_scalar_add(kT[:, do, half * 512:(half + 1) * 512], ps, bks[:, do:do + 1])
            # v projection (row-major): v[c, dv] = sum_di KTh[di, c] * Wv[di, dv] (+bv)
            for q in range(4):
                wt = wslice("wv", q, f32r)
                for cc4 in range(4):
                    cc = half * 4 + cc4
                    ps = ppa.tile([P, 512], f32, tag="acc")
                    for di in range(DT):
                        MM(ps[:, 0:256], KTh[:, di, cc4 * 128:(cc4 + 1) * 128], wt[:, di, :],
                           start=di == 0, stop=di == DT - 1)
                    h0 = q * 4
                    nc.vector.tensor_add(
                        v_sb[:, cc, h0:h0 + 4, 0:DH],
                        ps[:, 0:256].rearrange("p (h d) -> p h d", h=4),
                        bvb[:, h0 * DH:(h0 + 4) * DH].rearrange("p (h d) -> p h d", h=4),
                    )

        # ---- attention, head by head (interleaved E -> exp -> AV, bf16 operands)
        OT = act.tile([P, DT, R], bf16, tag="c16")       # -> sq (LN scratch)
        for h in range(H):
            t, po = h // 2, (h % 2) * DH
            qh = qT[po:po + DH, t, :]
            kh = kT[po:po + DH, t, :]
            po_t = ppa.tile([P, 512], f32, tag="acc")
            for g in range(4):
                pe_t = ppe.tile([P, 2, 512], f32, tag="e")
                for c2 in range(2):
                    cc = g * 2 + c2
                    MM(pe_t[:, c2, :], kh[:, cc * 128:(cc + 1) * 128], qh, start=True, stop=True)
                a_t = ap_.tile([P, 2, R], bf16, tag="aexp")
                nc.scalar.activation(a_t, pe_t, AF.Exp, bias=zeros_col, scale=SCALE)
                for c2 in range(2):
                    cc = g * 2 + c2
                    MM(po_t[0:DH + 1, :], v_sb[:, cc, h, :], a_t[:, c2, :],
                       start=cc == 0, stop=cc == DT - 1)
            sums = sm.tile([1, R], f32, tag="sums")
            nc.vector.tensor_copy(sums, po_t[DH:DH + 1, :])
            rec = sm.tile([1, R], f32, tag="rec")
            nc.vector.reciprocal(rec, sums)
            pb_t = ppb.tile([P, 512], f32, tag="pb")
            MM(pb_t[0:DH, :], ones_row[:, 0:DH], rec, start=True, stop=True)
            recb = sm.tile([DH, R], f32, tag="recb")
            nc.vector.tensor_copy(recb, pb_t[0:DH, :])
            nc.vector.tensor_mul(OT[po:po + DH, t, :], po_t[0:DH, :], recb)

        # ---- output projection + residual: X0 = Q + O @ Wo + bo
        X0 = act.tile([P, DT, R], f32, tag="b16")
        for q in range(4):
            wt = wslice("wo", q, bf16)
            for dof in range(2):
                do = q * 2 + dof
                ps = ppa.tile([P, 512], f32, tag="acc")
                for di in range(DT):
                    MM(ps, wt[:, di, dof * 128:(dof + 1) * 128], OT[:, di, :],
                       start=di == 0, stop=di == DT - 1)
                nc.vector.scalar_tensor_tensor(X0[:, do, :], ps, bos[:, do:do + 1],
                                               QT[:, do, :].bitcast(f32), OP.add, OP.add)

        # ---- layernorm (feature-major): stats over partitions via bf16 ones-matmul
        def layernorm(x_sb, g_t, be_t, out_sb):
            xb = tmp.tile([P, DT, R], bf16, tag="xb")
            nc.vector.tensor_copy(xb, x_sb)
            sq = act.tile([P, DT, R], bf16, tag="c16")
            nc.vector.tensor_mul(sq, x_sb, x_sb)
            psm = ppb.tile([P, 512], f32, tag="pb")
            for di in range(DT):
                MM(psm[0:1, :], ones_cb, xb[:, di, :], start=di == 0, stop=di == DT - 1)
            pss = ppb.tile([P, 512], f32, tag="pb")
            for di in range(DT):
                MM(pss[0:1, :], ones_cb, sq[:, di, :], start=di == 0, stop=di == DT - 1)
            mean = sm.tile([1, R], f32, tag="mean")
            nc.scalar.mul(mean, psm[0:1, :], 1.0 / D)
            msq = sm.tile([1, R], f32, tag="msq")
            nc.scalar.mul(msq, pss[0:1, :], 1.0 / D)
            m2 = sm.tile([1, R], f32, tag="m2")
            nc.vector.tensor_mul(m2, mean, mean)
            var = sm.tile([1, R], f32, tag="var")
            nc.vector.tensor_sub(var, msq, m2)
            std = sm.tile([1, R], f32, tag="std")
            nc.scalar.activation(std, var, AF.Sqrt, bias=eps_t, scale=1.0)
            rstd = sm.tile([1, R], f32, tag="rstd")
            nc.vector.reciprocal(rstd, std)
            pm = ppb.tile([P, 512], f32, tag="pb")
            MM(pm, ones_row, mean, start=True, stop=True)
            pr = ppb.tile([P, 512], f32, tag="pb")
            MM(pr, ones_row, rstd, start=True, stop=True)
            meanb = tmp.tile([P, R], f32, tag="meanb")
            nc.vector.tensor_copy(meanb, pm)
            rstdb = tmp.tile([P, R], f32, tag="rstdb")
            nc.vector.tensor_copy(rstdb, pr)
            for di in range(DT):
                t1 = tmp.tile([P, R], f32, tag="t1", bufs=2)
                nc.vector.tensor_sub(t1, x_sb[:, di, :], meanb)
                nc.vector.scalar_tensor_tensor(t1, t1, g_t[:, di:di + 1], rstdb, OP.mult, OP.mult)
                nc.vector.tensor_scalar_add(out_sb[:, di, :], t1, be_t[:, di:di + 1])

        XT = act.tile([P, DT, R], bf16, tag="a16")
        layernorm(X0, g0s, be0s, XT)

        # ---- FFN1: H = relu(X @ W1 + b1), bf16 compute, feature-major HT
        w1re = w1.rearrange("(t p) f -> p t f", p=P)
        HT = act.tile([P, FT, R], bf16, tag="kt16")
        for s in range(16):
            w1t = wp.tile([P, DT, 256], bf16, tag="w")
            nc.sync.dma_start(w1t, w1re[:, :, s * 256:(s + 1) * 256])
            for fc in range(2):
                ff = s * 2 + fc
                ps = ppa.tile([P, 512], f32, tag="acc")
                for di in range(DT):
                    MM(ps, w1t[:, di, fc * 128:(fc + 1) * 128], XT[:, di, :],
                       start=di == 0, stop=di == DT - 1)
                nc.scalar.activation(HT[:, ff, :], ps, AF.Relu, bias=b1s[:, ff:ff + 1], scale=1.0)

        # ---- FFN2 + residual: X1 = X + H @ W2 + b2
        w2re = w2.rearrange("(t p) f -> p t f", p=P)
        X1 = act.tile([P, DT, R], f32, tag="b16")
        for do in range(DT):
            ps = ppa.tile([P, 512], f32, tag="acc")
            for sh in range(2):
                w2t = wp.tile([P, FT // 2, 128], bf16, tag="w")
                nc.sync.dma_start(w2t, w2re[:, sh * 16:(sh + 1) * 16, do * 128:(do + 1) * 128])
                for f2 in range(FT // 2):
                    ff = sh * 16 + f2
                    MM(ps, w2t[:, f2, :], HT[:, ff, :], start=ff == 0, stop=ff == FT - 1)
            nc.vector.scalar_tensor_tensor(X1[:, do, :], ps, b2s[:, do:do + 1], XT[:, do, :], OP.add, OP.add)

        # ---- LN1 -> out
        OUTT = act.tile([P, DT, R], f32, tag="a16")
        layernorm(X1, g1s, be1s, OUTT)
        nc.sync.dma_start(out.rearrange("(t p) r -> p t r", p=P), OUTT)

    nc.compile()
    return nc


def _get_nc():
    if "nc" not in _CACHE:
        _CACHE["nc"] = _build()
    return _CACHE["nc"]


def _pack_vecs(inputs):
    f = lambda x: np.asarray(x, dtype=np.float32)
    cols = [f(inputs[k]).reshape(DT, P).T for k in
            ("bq", "bk", "bo", "b2", "g0", "be0", "g1", "be1")]
    cols.append(f(inputs["b1"]).reshape(FT, P).T)
    return np.ascontiguousarray(np.concatenate(cols, axis=1))


def kernel(**inputs):
    from concourse.bass_utils import run_bass_kernel_spmd

    nc = _get_nc()
    f = lambda x: np.ascontiguousarray(np.asarray(x, dtype=np.float32))
    fb = lambda x: np.ascontiguousarray(np.asarray(x, dtype=np.float32).astype(ml_dtypes.bfloat16))
    Q, K = f(inputs["Q"]), f(inputs["K"])
    shared = {
        "wq": f(inputs["Wq"]), "wk": f(inputs["Wk"]), "wv": f(inputs["Wv"]),
        "wo": fb(inputs["Wo"]), "w1": fb(inputs["W1"]), "w2": fb(inputs["W2"]),
        "bv": f(inputs["bv"]),
        "vecs": _pack_vecs(inputs),
    }
    kts = [np.ascontiguousarray(K[b].T) for b in range(B)]
    in_maps = []
    for core in range(NCORES):
        b, rh = core // 2, core % 2
        m = dict(shared)
        m["qt"] = np.ascontiguousarray(Q[b, rh * R:(rh + 1) * R, :].T)
        m["kt"] = kts[b]
        in_maps.append(m)
    res = run_bass_kernel_spmd(nc, in_maps, core_ids=list(range(NCORES)),
                               **_CACHE.get("run_kwargs", {}))
    _CACHE["last_result"] = res
    outp = np.empty((B, NQ, D), np.float32)
    for core in range(NCORES):
        b, rh = core // 2, core % 2
        outp[b, rh * R:(rh + 1) * R, :] = res.results[core]["out"].T
    return outp
